# revision 39
# baseline (speedup 1.0000x reference)
"""Paged causal GQA attention prefill on 8 Trainium2 NeuronCores.

Problem shape (hardcoded): H=32 query heads, KV=8 kv heads (GQA group 4),
D=128, S=128 new tokens, PAST=8064, T=8192 context, block_size=128,
128 physical cache blocks of which 64 logical blocks are live.

Sharding: tensor-parallel over KV heads — core h owns kv head h and its 4
query heads (512 packed query columns).

Kernel structure (bf16 data path, f32 PSUM accumulation). The scalar
engine's exp throughput (64*512 columns/lane at 1.2 GHz ~= 27us + per-
instruction overhead) is the fundamental bottleneck; everything else is
arranged to hide under it:
- Host gathers the paged cache through the block table, transposes K to
  [D, T] (block 63 first, then qT, then blocks 0..62) and packs V as
  [BS, NBLK*D], casts to bf16 (no on-chip transpose, half the HBM traffic).
- Three input DMA queues: K chunks on sync, V chunks on gpsimd, the three
  earliest K chunks (blocks 0..8) on the vector queue, so the exp stream
  starts ~2us after window-open and never starves.
- Dummy 256-col matmuls on memset tiles warm the PE out of its low
  p-state and a dummy exp pre-loads the ACT table, all under the DMA head.
- Block 63 (the only causally-masked block) is processed FIRST so the
  mask multiply sits in the pipeline ramp, not the tail.
- Scores: 3 blocks per batch, double-buffered (2x3 PSUM banks + out +
  warm = 8). Larger single-buffered batches serialize
  exp(k) -> scores(k+1) -> exp(k+1).
- Software-pipelined emission: batch b's scores and exp are emitted
  BEFORE batch b-1's PV matmuls so the PE never delays the next exp.
- Softmax denominator fully on-device: probs batches accumulate on the
  DVE in bf16 (2x_1P) into accA (odd batches) / accB (even batches +
  block 63's masked probs); once each accumulator completes, 3 ones^T @
  acc matmuls on the PE fold its column sums into warm_ps row 0, and the
  last two batches' probs are folded the same way (9 matmuls, one
  accumulation group). Only outT [128,512] + den [1,512] leave the
  device (~132KB instead of ~1.2MB) so the final DMA drain is short.
- outT is cast on the DVE and DMA'd as two partition halves on the sync
  and gpsimd queues in parallel; den is copied + DMA'd by the (by then
  idle) scalar engine.
- No max-subtraction: |scores*scale| <~ 8 so exp is safe in f32.
"""

import os
import sys

if "/opt/trn_rl_repo" not in sys.path:
    sys.path.insert(0, "/opt/trn_rl_repo")

import numpy as np

H, D, KV, S, PAST, BS, NB = 32, 128, 8, 128, 8064, 128, 128
T = PAST + S  # 8192
NBLK = T // BS  # 64
G = H // KV  # 4
SP = G * S  # 512 packed query columns per core
AB = 3  # blocks per act batch
# PE p-state warmup matmuls under the DMA head.  The HAM clock-gate flips
# to 2.4 GHz after ~3.4us of CONTINUOUS PE activity; chunk 0 arrives ~2.4us
# after the earliest possible warm start, so the best tradeoff is: start
# warm matmuls as early as possible (wsrcA memset on gpsimd right behind
# the framework's const memsets, first matmuls at N=128 on wsrcA alone so
# they don't wait for wsrcB) and END right at chunk-0-ready — the flip then
# lands one batch into the real stream instead of delaying it.
# Measured: warm starts ~1.0us in-window via the gpsimd memset, chunk 0 is
# ready ~4.6us — so ~3.6us of warm matmuls both spans the full 3.4us HAM
# window (flip at ~4.4us, BEFORE the first scores) and ends right at
# chunk-0-ready.  Too few leaves a hole that resets the window and the
# whole ramp runs at 1.2 GHz.
N_WARM_SMALL = 4  # N=128, wsrcA x wsrcA
N_WARM_MM = 15  # N=256, wsrcA x wsrcB
WARM_N = 256  # columns per warmup matmul

# act batches: block 63 alone first, then 3s over blocks 0..62 (the tail
# den matmuls are deferred past PV(62), so a full final batch streams at
# the steady cadence instead of two short stalling ones)
BATCHES = [(63, 1)] + [(lo, 3) for lo in range(0, 63, 3)]
NBATCH = len(BATCHES)  # 22

# ktT column layout: [K(63) | qT | K(0) .. K(62)]  (so the first chunk —
# block 63's K plus qT, one contiguous 640-col slice — gates the first
# matmul alone).  Block b (b<63) starts at col 640 + 128*b.
KTW = BS + SP + 63 * BS  # 8704


def _kcol(b):
    return 0 if b == 63 else BS + SP + BS * b


# K/V DMA chunk ladders (block_start, n_blocks), sized so each chunk's
# completion (issue slot ~0.65us + transfer at ~125 GB/s/queue + ~0.6us
# sem prop) lands just before its first consuming batch.  qT rides the
# scalar engine's HWDGE queue first (it gates every scores matmul), the
# earliest K chunks follow it; block 63's K is a tiny solo chunk on sync
# so its LDWEIGHTS can preload while qT is still in flight.
SYNC_K = [(3, 3), (6, 3), (9, 3), (12, 6), (18, 12), (30, 12), (42, 12), (54, 6), (60, 3)]
GPS_V = [(62, 2), (0, 3), (3, 3), (6, 6), (12, 12), (24, 12), (36, 12), (48, 9), (57, 5)]

# merged output layout [128, 1024] bf16: outT(512) | den(512, row 0 only)
O_OUT, O_DEN = 0, SP
O_W = 2 * SP

_cache: dict = {}
last_exec_time_ns = None
last_profile = None


def _build(scale):
    from concourse import bacc, mybir
    import concourse.tile as tile

    F32 = mybir.dt.float32
    BF16 = mybir.dt.bfloat16
    EXP = mybir.ActivationFunctionType.Exp

    nc = bacc.Bacc(None, target_bir_lowering=False)

    ktT = nc.declare_dram_parameter("ktT", [D, KTW], BF16, isOutput=False)
    vpk = nc.declare_dram_parameter("vpk", [BS, NBLK * D], BF16, isOutput=False)
    mask_in = nc.declare_dram_parameter("mask_in", [BS, SP], BF16, isOutput=False)
    outO = nc.declare_dram_parameter("outO", [BS, O_W], BF16, isOutput=True)

    with tile.TileContext(nc) as tc:
        with (
            tc.sbuf_pool(name="cst", bufs=1) as cst,
            tc.sbuf_pool(name="kin", bufs=1) as kin,
            tc.sbuf_pool(name="vin", bufs=1) as vin,
            tc.sbuf_pool(name="prb", bufs=4) as prb,
            tc.psum_pool(name="scp", bufs=2) as scp,
            tc.psum_pool(name="acc", bufs=1) as acc,
            tc.psum_pool(name="pwm", bufs=1) as pwm,
        ):
            # --- head: warm the PE + ACT table while DMAs stream ---------
            # chunk 0 on sync = [K(63) | qT | K(0..2)], one contiguous
            # 1024-col transfer (per-transfer latency has a ~2.5us fixed
            # floor, so one joint chunk beats several small ones and feeds
            # the first four batches at once)
            kmap = {}
            c0_sb = kin.tile([D, 8 * BS], BF16, tag="k63")
            nc.sync.dma_start(c0_sb[:], ktT[:, 0 : 8 * BS])
            qT_ap = c0_sb[:, BS : BS + SP]
            kmap[63] = (c0_sb, 0)
            for j in range(3):
                kmap[j] = (c0_sb, BS + SP + j * BS)
            # wsrcA memset on gpsimd (lands right after the framework const
            # memsets), wsrcB on vector in parallel; the first warm matmuls
            # only need wsrcA.
            wsrcA = cst.tile([128, 128], BF16)
            nc.gpsimd.memset(wsrcA[:], 1.0)
            wsrcB = cst.tile([128, WARM_N], BF16)
            nc.vector.memset(wsrcB[:], 0.5)
            warm_ps = pwm.tile([128, SP], F32)
            for r in range(N_WARM_SMALL):
                nc.tensor.matmul(
                    warm_ps[:, 0:BS], wsrcA[:], wsrcA[:], start=True, stop=True
                )
            for r in range(N_WARM_MM):
                nc.tensor.matmul(
                    warm_ps[:, 0:WARM_N], wsrcA[:], wsrcB[:], start=True, stop=True
                )
            warm_sb = cst.tile([128, 8], BF16)
            nc.scalar.activation(warm_sb[:], wsrcB[:, 0:8], EXP, scale=1.0)

            # --- input DMA ladders ---------------------------------------
            # K slices: block b -> (sbuf tile, col offset)
            for s, n in SYNC_K:
                c0 = _kcol(s)
                k_sb = kin.tile([D, n * BS], BF16, tag=f"k{s}")
                nc.sync.dma_start(k_sb[:], ktT[:, c0 : c0 + n * BS])
                for j in range(n):
                    kmap[s + j] = (k_sb, j * BS)

            mask_sb = cst.tile([BS, SP], BF16)
            nc.gpsimd.dma_start(mask_sb[:], mask_in[:])
            vmap = {}
            for s, n in GPS_V:
                v_sb = vin.tile([BS, n * D], BF16, tag=f"v{s}")
                nc.gpsimd.dma_start(v_sb[:], vpk[:, s * D : (s + n) * D])
                for j in range(n):
                    vmap[s + j] = (v_sb, j * D)

            accA_sb = cst.tile([BS, AB * SP], BF16)
            accB_sb = cst.tile([BS, AB * SP], BF16)
            pm_sb = cst.tile([BS, SP], BF16)

            out_ps = acc.tile([D, SP], F32)

            # den accumulation group on warm_ps row 0: 3 ones^T@accA (after
            # b19) + 3 ones^T@accB (after b20) + 3 over the last batches'
            # probs.  PE column-sums in f32 == the host-side sums they replace.
            def den_mm(rhs_ap, start, stop):
                nc.tensor.matmul(
                    warm_ps[0:1, :],
                    wsrcA[:, 0:1],
                    rhs_ap,
                    start=start,
                    stop=stop,
                    skip_group_check=True,
                )

            # --- consume stage: mask/PV/denominator for a finished batch --
            # accB takes even batches 2..18 (+ masked block 63) and closes
            # at b18; accA takes odd batches 1..19 plus b20 and closes at
            # b20 — both column-sum folds (3 ones^T matmuls each) land on
            # the PE while exps for b21/b22 are still streaming, so the
            # only post-last-exp den work is the last three blocks' own
            # matmuls, emitted AFTER PV(62) to unblock the output cast.
            tail_den = []

            def consume(b, lo, n, probs_sb):
                for j in range(n):
                    i = lo + j
                    p = probs_sb[:, j * SP : (j + 1) * SP]
                    if i == NBLK - 1:
                        nc.vector.tensor_mul(pm_sb[:], p, mask_sb[:])
                        p = pm_sb[:]
                    v_sb, vo = vmap[i]
                    nc.tensor.matmul(
                        out_ps[:],
                        v_sb[:, vo : vo + D],
                        p,
                        start=(b == 0),
                        stop=(i == 62),  # block 62 is processed last
                        skip_group_check=True,
                    )
                if b == NBATCH - 1:
                    # last batch (60,3): remember its probs; den matmuls
                    # are emitted after the final PV
                    for j in range(n):
                        tail_den.append(probs_sb[:, j * SP : (j + 1) * SP])
                elif b == 0:
                    pass  # pm_sb folded into accB below (b == 2)
                elif b % 2 == 1 or b == 20:
                    if b == 1:
                        nc.vector.tensor_copy(accA_sb[:], probs_sb[:])
                    else:
                        nc.vector.tensor_add(accA_sb[:], accA_sb[:], probs_sb[:])
                    if b == 20:  # accA complete -> fold after the last PVs
                        for j in range(AB):
                            tail_den.append(accA_sb[:, j * SP : (j + 1) * SP])
                else:
                    if b == 2:
                        nc.vector.tensor_copy(accB_sb[:], probs_sb[:])
                        nc.vector.tensor_add(
                            accB_sb[:, 0:SP], accB_sb[:, 0:SP], pm_sb[:]
                        )
                    else:
                        nc.vector.tensor_add(accB_sb[:], accB_sb[:], probs_sb[:])
                    if b == 18:  # accB complete -> fold column sums on PE
                        for j in range(AB):
                            den_mm(
                                accB_sb[:, j * SP : (j + 1) * SP],
                                start=(j == 0),
                                stop=False,
                            )

            # --- main loop, software-pipelined (consume lags 2 batches so
            # PV matmuls never sit between the PE's score groups).  Extra
            # warmup matmuls are interleaved after the first batches so the
            # PE's HAM activity window never sees a >1us hole during the
            # DMA-gated ramp (otherwise it stays at 1.2 GHz into the loop).
            WARM_EXTRA = {0: 3, 1: 2, 2: 2, 3: 1}
            pending = []
            for b, (lo, n) in enumerate(BATCHES):
                sc_ps = scp.tile([128, AB * SP], F32, tag="sc")
                for j in range(n):
                    i = lo + j
                    k_sb, ko = kmap[i]
                    nc.tensor.matmul(
                        sc_ps[:, j * SP : (j + 1) * SP],
                        k_sb[:, ko : ko + BS],
                        qT_ap,
                        start=True,
                        stop=True,
                    )
                probs_sb = prb.tile([128, AB * SP], BF16, tag="probs")
                if b == NBATCH - 1:
                    # last batch: one ACTIVATE per block so PV(60) can start
                    # after the first third of the exp instead of the whole
                    # 1536 columns — pulls the output-cast chain ~1us earlier
                    for j in range(n):
                        nc.scalar.activation(
                            probs_sb[:, j * SP : (j + 1) * SP],
                            sc_ps[:, j * SP : (j + 1) * SP],
                            EXP,
                            scale=scale,
                        )
                else:
                    nc.scalar.activation(
                        probs_sb[:, 0 : n * SP], sc_ps[:, 0 : n * SP], EXP, scale=scale
                    )
                for _ in range(WARM_EXTRA.get(b, 0)):
                    nc.tensor.matmul(
                        warm_ps[:, 0:WARM_N], wsrcA[:], wsrcB[:], start=True, stop=True
                    )
                pending.append((b, lo, n, probs_sb))
                # lag 3 (prb has 4 buffers): the last scores batch is emitted
                # ahead of two batches of PV matmuls, so exp(b21) isn't gated
                # behind them in the PE queue
                if len(pending) > 3:
                    consume(*pending.pop(0))
            for args in pending:
                consume(*args)
            # den matmuls for blocks 60..62, after PV(62) so the output
            # cast isn't stuck behind them on the PE
            for t, p_ap in enumerate(tail_den):
                den_mm(p_ap, start=False, stop=(t == len(tail_den) - 1))

            # --- tail: outT on DVE + sync DMA; den copied + DMA'd by the
            # (by then idle) scalar engine.  Nothing late on the gpsimd
            # queue so its expensive SWDGE drain overlaps the mid-loop
            # instead of extending the tail.
            o_sb = cst.tile([D, SP], BF16)
            nc.vector.tensor_copy(o_sb[:], out_ps[:])
            nc.sync.dma_start(outO[:, O_OUT : O_OUT + SP], o_sb[:])
            den_sb = cst.tile([1, SP], BF16)
            nc.scalar.copy(den_sb[:], warm_ps[0:1, :])
            # den on the scalar queue: its 64B transfer drains in parallel
            # with the sync queue's outT transfer
            nc.scalar.dma_start(outO[0:1, O_DEN : O_DEN + SP], den_sb[:])

    nc.finalize()
    return nc


def _install_ntff_hook():
    """antenv.axon_hooks is absent on this image; inject it and register the
    ctypes-based NTFF profile hook so run_bass_kernel_spmd(trace=True) works."""
    import types

    if "antenv.axon_hooks" in sys.modules:
        return
    mod = types.ModuleType("antenv.axon_hooks")
    state = {"hook": None}
    mod.set_axon_ntff_profile_hook = lambda h: state.__setitem__("hook", h)
    mod.get_axon_ntff_profile_hook = lambda: state["hook"]
    sys.modules["antenv.axon_hooks"] = mod
    try:
        import antenv

        antenv.axon_hooks = mod
    except ImportError:
        pass
    try:
        from trn_agent_boot.trn_boot import _ntff_profile_via_ctypes

        mod.set_axon_ntff_profile_hook(
            _ntff_profile_via_ctypes("/opt/axon/libaxon_pjrt.so")
        )
    except Exception as e:  # degrade to no-trace
        print(f"NTFF hook registration failed: {e}")


def kernel(
    query_state,
    key_state,
    value_state,
    attn_mask,
    past_key_state,
    past_value_state,
    seq_position,
    scale,
    block_tables,
    block_size,
    **_ignored,
):
    global last_exec_time_ns, last_profile
    from concourse.bass_utils import run_bass_kernel_spmd
    import ml_dtypes

    bf16 = ml_dtypes.bfloat16

    q = np.asarray(query_state, dtype=np.float32)
    k = np.asarray(key_state, dtype=np.float32)
    v = np.asarray(value_state, dtype=np.float32)
    pk = np.asarray(past_key_state, dtype=np.float32)
    pv = np.asarray(past_value_state, dtype=np.float32)
    bt = tuple(int(x) for x in np.asarray(block_tables).tolist())
    scale_f = float(np.asarray(scale))
    sp = int(np.asarray(seq_position))
    bs = int(np.asarray(block_size))

    assert q.shape == (1, H, S, D) and pk.shape == (NB, KV, BS, D)
    assert sp == PAST and bs == BS and len(bt) == NBLK

    key = (scale_f,)
    nc = _cache.get(key)
    if nc is None:
        nc = _build(scale_f)
        _cache.clear()
        _cache[key] = nc

    mseq = (
        np.arange(BS, dtype=np.int32)[:, None] <= np.arange(S, dtype=np.int32)[None, :]
    ).astype(np.float32)
    mask = np.tile(mseq, (1, G)).astype(bf16)  # [j, g*128+s]

    qg = q[0].reshape(KV, G, S, D)
    bt_arr = np.asarray(bt[: NBLK - 1], dtype=np.int64)
    # host-side gather: context blocks in logical order [NBLK, KV, BS, D];
    # the new K/V exactly overwrite logical block 63 (seq_position == 63 * BS)
    kctx = np.concatenate([pk[bt_arr], k[0][None]], axis=0)
    vctx = np.concatenate([pv[bt_arr], v[0][None]], axis=0)
    in_maps = []
    for h in range(KV):
        # ktT[d, blk*BS+j] : K transposed, logical token order
        ktT_h = kctx[:, h].transpose(2, 0, 1).reshape(D, T).astype(bf16)
        # vpk[j, blk*D+d] : V with in-block token index on partitions
        vpk_h = np.ascontiguousarray(
            vctx[:, h].transpose(1, 0, 2).reshape(BS, NBLK * D).astype(bf16)
        )
        qT_h = qg[h].transpose(2, 0, 1).reshape(D, SP).astype(bf16)
        # column order [K(63) | qT | K(0..62)]
        ktq_h = np.ascontiguousarray(
            np.concatenate(
                [ktT_h[:, 63 * BS :], qT_h, ktT_h[:, : 63 * BS]], axis=1
            )
        )
        in_maps.append({"ktT": ktq_h, "vpk": vpk_h, "mask_in": mask})

    trace = bool(int(os.environ.get("BASS_ATTN_TRACE", "0")))
    if trace:
        _install_ntff_hook()
    res = run_bass_kernel_spmd(nc, in_maps, core_ids=list(range(KV)), trace=trace)
    last_exec_time_ns = res.exec_time_ns
    last_profile = res

    out = np.empty((1, S, H * D), dtype=np.float32)
    for h in range(KV):
        oo = res.results[h]["outO"].astype(np.float32)  # [128, O_W]
        oT = oo[:, O_OUT : O_OUT + SP]  # [d, g*128+s], unnormalized
        den = oo[0, O_DEN : O_DEN + SP]  # [g*128+s]
        o = (oT / den[None, :]).reshape(D, G, S).transpose(2, 1, 0)  # [s, g, d]
        out[0, :, h * G * D : (h + 1) * G * D] = o.reshape(S, G * D)
    return out


# revision 40
# speedup vs baseline: 1.0326x; 1.0326x over previous
"""Paged causal GQA attention prefill on 8 Trainium2 NeuronCores.

Problem shape (hardcoded): H=32 query heads, KV=8 kv heads (GQA group 4),
D=128, S=128 new tokens, PAST=8064, T=8192 context, block_size=128,
128 physical cache blocks of which 64 logical blocks are live.

Sharding: tensor-parallel over KV heads — core h owns kv head h and its 4
query heads (512 packed query columns).

Kernel structure (bf16 data path, f32 PSUM accumulation). The scalar
engine's exp throughput (64*512 columns/lane at 1.2 GHz ~= 27us + per-
instruction overhead) is the fundamental bottleneck; everything else is
arranged to hide under it:
- Host gathers the paged cache through the block table, transposes K to
  [D, T] (block 63 first, then qT, then blocks 0..62) and packs V as
  [BS, NBLK*D], casts to bf16 (no on-chip transpose, half the HBM traffic).
- Three input DMA queues: K chunks on sync, V chunks on gpsimd, the three
  earliest K chunks (blocks 0..8) on the vector queue, so the exp stream
  starts ~2us after window-open and never starves.
- Dummy 256-col matmuls on memset tiles warm the PE out of its low
  p-state and a dummy exp pre-loads the ACT table, all under the DMA head.
- Block 63 (the only causally-masked block) is processed FIRST so the
  mask multiply sits in the pipeline ramp, not the tail.
- Scores: 3 blocks per batch, double-buffered (2x3 PSUM banks + out +
  warm = 8). Larger single-buffered batches serialize
  exp(k) -> scores(k+1) -> exp(k+1).
- Software-pipelined emission: batch b's scores and exp are emitted
  BEFORE batch b-1's PV matmuls so the PE never delays the next exp.
- Softmax denominator fully on-device: probs batches accumulate on the
  DVE in bf16 (2x_1P) into accA (odd batches) / accB (even batches +
  block 63's masked probs); once each accumulator completes, 3 ones^T @
  acc matmuls on the PE fold its column sums into warm_ps row 0, and the
  last two batches' probs are folded the same way (9 matmuls, one
  accumulation group). Only outT [128,512] + den [1,512] leave the
  device (~132KB instead of ~1.2MB) so the final DMA drain is short.
- outT is cast on the DVE and DMA'd as two partition halves on the sync
  and gpsimd queues in parallel; den is copied + DMA'd by the (by then
  idle) scalar engine.
- No max-subtraction: |scores*scale| <~ 8 so exp is safe in f32.
"""

import os
import sys

if "/opt/trn_rl_repo" not in sys.path:
    sys.path.insert(0, "/opt/trn_rl_repo")

import numpy as np

H, D, KV, S, PAST, BS, NB = 32, 128, 8, 128, 8064, 128, 128
T = PAST + S  # 8192
NBLK = T // BS  # 64
G = H // KV  # 4
SP = G * S  # 512 packed query columns per core
AB = 3  # blocks per act batch
# PE p-state warmup matmuls under the DMA head.  The HAM clock-gate flips
# to 2.4 GHz after ~3.4us of CONTINUOUS PE activity; chunk 0 arrives ~2.4us
# after the earliest possible warm start, so the best tradeoff is: start
# warm matmuls as early as possible (wsrcA memset on gpsimd right behind
# the framework's const memsets, first matmuls at N=128 on wsrcA alone so
# they don't wait for wsrcB) and END right at chunk-0-ready — the flip then
# lands one batch into the real stream instead of delaying it.
# Measured: warm starts ~1.0us in-window via the gpsimd memset, chunk 0 is
# ready ~4.6us — so ~3.6us of warm matmuls both spans the full 3.4us HAM
# window (flip at ~4.4us, BEFORE the first scores) and ends right at
# chunk-0-ready.  Too few leaves a hole that resets the window and the
# whole ramp runs at 1.2 GHz.
N_WARM_SMALL = 4  # N=128, wsrcA x wsrcA
N_WARM_MM = 15  # N=256, wsrcA x wsrcB
WARM_N = 256  # columns per warmup matmul

# act batches: block 63 alone first, then 3s over blocks 0..62 (the tail
# den matmuls are deferred past PV(62), so a full final batch streams at
# the steady cadence instead of two short stalling ones)
BATCHES = [(63, 1)] + [(lo, 3) for lo in range(0, 63, 3)]
NBATCH = len(BATCHES)  # 22

# ktT column layout: [K(63) | qT | K(0) .. K(62)]  (so the first chunk —
# block 63's K plus qT, one contiguous 640-col slice — gates the first
# matmul alone).  Block b (b<63) starts at col 640 + 128*b.
KTW = BS + SP + 63 * BS  # 8704


def _kcol(b):
    return 0 if b == 63 else BS + SP + BS * b


# K/V DMA chunk ladders (block_start, n_blocks), sized so each chunk's
# completion (issue slot ~0.65us + transfer at ~125 GB/s/queue + ~0.6us
# sem prop) lands just before its first consuming batch.  qT rides the
# scalar engine's HWDGE queue first (it gates every scores matmul), the
# earliest K chunks follow it; block 63's K is a tiny solo chunk on sync
# so its LDWEIGHTS can preload while qT is still in flight.
SYNC_K = [(3, 3), (6, 3), (9, 3), (12, 6), (18, 12), (30, 12), (42, 12), (54, 6), (60, 3)]
GPS_V = [(62, 2), (0, 3), (3, 3), (6, 6), (12, 12), (24, 12), (36, 12), (48, 9), (57, 5)]

# merged output layout [128, 1024] bf16: outT(512) | den(512, row 0 only)
O_OUT, O_DEN = 0, SP
O_W = 2 * SP

_cache: dict = {}
last_exec_time_ns = None
last_profile = None


def _build(scale):
    from concourse import bacc, mybir
    import concourse.tile as tile

    F32 = mybir.dt.float32
    BF16 = mybir.dt.bfloat16
    EXP = mybir.ActivationFunctionType.Exp

    nc = bacc.Bacc(None, target_bir_lowering=False)

    ktT = nc.declare_dram_parameter("ktT", [D, KTW], BF16, isOutput=False)
    vpk = nc.declare_dram_parameter("vpk", [BS, NBLK * D], BF16, isOutput=False)
    mask_in = nc.declare_dram_parameter("mask_in", [BS, SP], BF16, isOutput=False)
    outO = nc.declare_dram_parameter("outO", [BS, O_W], BF16, isOutput=True)

    with tile.TileContext(nc) as tc:
        with (
            tc.sbuf_pool(name="cst", bufs=1) as cst,
            tc.sbuf_pool(name="kin", bufs=1) as kin,
            tc.sbuf_pool(name="vin", bufs=1) as vin,
            tc.sbuf_pool(name="prb", bufs=4) as prb,
            tc.psum_pool(name="scp", bufs=2) as scp,
            tc.psum_pool(name="acc", bufs=1) as acc,
            tc.psum_pool(name="pwm", bufs=1) as pwm,
        ):
            # --- head: warm the PE + ACT table while DMAs stream ---------
            # chunk 0 on sync = [K(63) | qT | K(0..2)], one contiguous
            # 1024-col transfer (per-transfer latency has a ~2.5us fixed
            # floor, so one joint chunk beats several small ones and feeds
            # the first four batches at once)
            kmap = {}
            c0_sb = kin.tile([D, 8 * BS], BF16, tag="k63")
            nc.sync.dma_start(c0_sb[:], ktT[:, 0 : 8 * BS])
            qT_ap = c0_sb[:, BS : BS + SP]
            kmap[63] = (c0_sb, 0)
            for j in range(3):
                kmap[j] = (c0_sb, BS + SP + j * BS)
            # wsrcA memset on gpsimd (lands right after the framework const
            # memsets), wsrcB on vector in parallel; the first warm matmuls
            # only need wsrcA.
            wsrcA = cst.tile([128, 128], BF16)
            nc.gpsimd.memset(wsrcA[:], 1.0)
            wsrcB = cst.tile([128, WARM_N], BF16)
            nc.vector.memset(wsrcB[:], 0.5)
            warm_ps = pwm.tile([128, SP], F32)
            for r in range(N_WARM_SMALL):
                nc.tensor.matmul(
                    warm_ps[:, 0:BS], wsrcA[:], wsrcA[:], start=True, stop=True
                )
            for r in range(N_WARM_MM):
                nc.tensor.matmul(
                    warm_ps[:, 0:WARM_N], wsrcA[:], wsrcB[:], start=True, stop=True
                )
            warm_sb = cst.tile([128, 8], BF16)
            nc.scalar.activation(warm_sb[:], wsrcB[:, 0:8], EXP, scale=1.0)

            # --- input DMA ladders ---------------------------------------
            # K slices: block b -> (sbuf tile, col offset)
            for s, n in SYNC_K:
                c0 = _kcol(s)
                k_sb = kin.tile([D, n * BS], BF16, tag=f"k{s}")
                nc.sync.dma_start(k_sb[:], ktT[:, c0 : c0 + n * BS])
                for j in range(n):
                    kmap[s + j] = (k_sb, j * BS)

            mask_sb = cst.tile([BS, SP], BF16)
            nc.gpsimd.dma_start(mask_sb[:], mask_in[:])
            vmap = {}
            for s, n in GPS_V:
                v_sb = vin.tile([BS, n * D], BF16, tag=f"v{s}")
                nc.gpsimd.dma_start(v_sb[:], vpk[:, s * D : (s + n) * D])
                for j in range(n):
                    vmap[s + j] = (v_sb, j * D)

            accA_sb = cst.tile([BS, AB * SP], BF16)
            accB_sb = cst.tile([BS, AB * SP], BF16)
            pm_sb = cst.tile([BS, SP], BF16)

            out_ps = acc.tile([D, SP], F32)

            # den accumulation group on warm_ps row 0: 3 ones^T@accA (after
            # b19) + 3 ones^T@accB (after b20) + 3 over the last batches'
            # probs.  PE column-sums in f32 == the host-side sums they replace.
            def den_mm(rhs_ap, start, stop):
                nc.tensor.matmul(
                    warm_ps[0:1, :],
                    wsrcA[:, 0:1],
                    rhs_ap,
                    start=start,
                    stop=stop,
                    skip_group_check=True,
                )

            # --- consume stage: mask/PV/denominator for a finished batch --
            # accB takes even batches 2..18 (+ masked block 63) and closes
            # at b18; accA takes odd batches 1..19 plus b20 and closes at
            # b20 — both column-sum folds (3 ones^T matmuls each) land on
            # the PE while exps for b21/b22 are still streaming, so the
            # only post-last-exp den work is the last three blocks' own
            # matmuls, emitted AFTER PV(62) to unblock the output cast.
            tail_den = []

            def consume(b, lo, n, probs_sb):
                for j in range(n):
                    i = lo + j
                    p = probs_sb[:, j * SP : (j + 1) * SP]
                    if i == NBLK - 1:
                        nc.vector.tensor_mul(pm_sb[:], p, mask_sb[:])
                        p = pm_sb[:]
                    v_sb, vo = vmap[i]
                    nc.tensor.matmul(
                        out_ps[:],
                        v_sb[:, vo : vo + D],
                        p,
                        start=(b == 0),
                        stop=(i == 62),  # block 62 is processed last
                        skip_group_check=True,
                    )
                if b == NBATCH - 1:
                    # last batch (60,3): remember its probs; den matmuls
                    # are emitted after the final PV
                    for j in range(n):
                        tail_den.append(probs_sb[:, j * SP : (j + 1) * SP])
                elif b == 0:
                    pass  # pm_sb folded into accB below (b == 2)
                elif b % 2 == 1 or b == 20:
                    if b == 1:
                        nc.vector.tensor_copy(accA_sb[:], probs_sb[:])
                    else:
                        nc.vector.tensor_add(accA_sb[:], accA_sb[:], probs_sb[:])
                    if b == 20:  # accA complete -> fold after the last PVs
                        for j in range(AB):
                            tail_den.append(accA_sb[:, j * SP : (j + 1) * SP])
                else:
                    if b == 2:
                        nc.vector.tensor_copy(accB_sb[:], probs_sb[:])
                        nc.vector.tensor_add(
                            accB_sb[:, 0:SP], accB_sb[:, 0:SP], pm_sb[:]
                        )
                    else:
                        nc.vector.tensor_add(accB_sb[:], accB_sb[:], probs_sb[:])
                    if b == 18:  # accB complete -> fold column sums on PE
                        for j in range(AB):
                            den_mm(
                                accB_sb[:, j * SP : (j + 1) * SP],
                                start=(j == 0),
                                stop=False,
                            )

            # --- main loop, software-pipelined (consume lags 2 batches so
            # PV matmuls never sit between the PE's score groups).  Extra
            # warmup matmuls are interleaved after the first batches so the
            # PE's HAM activity window never sees a >1us hole during the
            # DMA-gated ramp (otherwise it stays at 1.2 GHz into the loop).
            WARM_EXTRA = {0: 3, 1: 2, 2: 2, 3: 1}
            pending = []
            for b, (lo, n) in enumerate(BATCHES):
                sc_ps = scp.tile([128, AB * SP], F32, tag="sc")
                for j in range(n):
                    i = lo + j
                    k_sb, ko = kmap[i]
                    nc.tensor.matmul(
                        sc_ps[:, j * SP : (j + 1) * SP],
                        k_sb[:, ko : ko + BS],
                        qT_ap,
                        start=True,
                        stop=True,
                    )
                probs_sb = prb.tile([128, AB * SP], BF16, tag="probs")
                nc.scalar.activation(
                    probs_sb[:, 0 : n * SP], sc_ps[:, 0 : n * SP], EXP, scale=scale
                )
                for _ in range(WARM_EXTRA.get(b, 0)):
                    nc.tensor.matmul(
                        warm_ps[:, 0:WARM_N], wsrcA[:], wsrcB[:], start=True, stop=True
                    )
                pending.append((b, lo, n, probs_sb))
                if len(pending) > 2:
                    consume(*pending.pop(0))
            for args in pending:
                consume(*args)
            # den matmuls for blocks 60..62, after PV(62) so the output
            # cast isn't stuck behind them on the PE
            for t, p_ap in enumerate(tail_den):
                den_mm(p_ap, start=False, stop=(t == len(tail_den) - 1))

            # --- tail: outT on DVE + sync DMA; den copied + DMA'd by the
            # (by then idle) scalar engine.  Nothing late on the gpsimd
            # queue so its expensive SWDGE drain overlaps the mid-loop
            # instead of extending the tail.
            o_sb = cst.tile([D, SP], BF16)
            nc.vector.tensor_copy(o_sb[:], out_ps[:])
            nc.sync.dma_start(outO[:, O_OUT : O_OUT + SP], o_sb[:])
            den_sb = cst.tile([1, SP], BF16)
            nc.scalar.copy(den_sb[:], warm_ps[0:1, :])
            # den on the scalar queue: its 64B transfer drains in parallel
            # with the sync queue's outT transfer
            nc.scalar.dma_start(outO[0:1, O_DEN : O_DEN + SP], den_sb[:])

    nc.finalize()
    return nc


def _install_ntff_hook():
    """antenv.axon_hooks is absent on this image; inject it and register the
    ctypes-based NTFF profile hook so run_bass_kernel_spmd(trace=True) works."""
    import types

    if "antenv.axon_hooks" in sys.modules:
        return
    mod = types.ModuleType("antenv.axon_hooks")
    state = {"hook": None}
    mod.set_axon_ntff_profile_hook = lambda h: state.__setitem__("hook", h)
    mod.get_axon_ntff_profile_hook = lambda: state["hook"]
    sys.modules["antenv.axon_hooks"] = mod
    try:
        import antenv

        antenv.axon_hooks = mod
    except ImportError:
        pass
    try:
        from trn_agent_boot.trn_boot import _ntff_profile_via_ctypes

        mod.set_axon_ntff_profile_hook(
            _ntff_profile_via_ctypes("/opt/axon/libaxon_pjrt.so")
        )
    except Exception as e:  # degrade to no-trace
        print(f"NTFF hook registration failed: {e}")


def kernel(
    query_state,
    key_state,
    value_state,
    attn_mask,
    past_key_state,
    past_value_state,
    seq_position,
    scale,
    block_tables,
    block_size,
    **_ignored,
):
    global last_exec_time_ns, last_profile
    from concourse.bass_utils import run_bass_kernel_spmd
    import ml_dtypes

    bf16 = ml_dtypes.bfloat16

    q = np.asarray(query_state, dtype=np.float32)
    k = np.asarray(key_state, dtype=np.float32)
    v = np.asarray(value_state, dtype=np.float32)
    pk = np.asarray(past_key_state, dtype=np.float32)
    pv = np.asarray(past_value_state, dtype=np.float32)
    bt = tuple(int(x) for x in np.asarray(block_tables).tolist())
    scale_f = float(np.asarray(scale))
    sp = int(np.asarray(seq_position))
    bs = int(np.asarray(block_size))

    assert q.shape == (1, H, S, D) and pk.shape == (NB, KV, BS, D)
    assert sp == PAST and bs == BS and len(bt) == NBLK

    key = (scale_f,)
    nc = _cache.get(key)
    if nc is None:
        nc = _build(scale_f)
        _cache.clear()
        _cache[key] = nc

    mseq = (
        np.arange(BS, dtype=np.int32)[:, None] <= np.arange(S, dtype=np.int32)[None, :]
    ).astype(np.float32)
    mask = np.tile(mseq, (1, G)).astype(bf16)  # [j, g*128+s]

    qg = q[0].reshape(KV, G, S, D)
    bt_arr = np.asarray(bt[: NBLK - 1], dtype=np.int64)
    # host-side gather: context blocks in logical order [NBLK, KV, BS, D];
    # the new K/V exactly overwrite logical block 63 (seq_position == 63 * BS)
    kctx = np.concatenate([pk[bt_arr], k[0][None]], axis=0)
    vctx = np.concatenate([pv[bt_arr], v[0][None]], axis=0)
    in_maps = []
    for h in range(KV):
        # ktT[d, blk*BS+j] : K transposed, logical token order
        ktT_h = kctx[:, h].transpose(2, 0, 1).reshape(D, T).astype(bf16)
        # vpk[j, blk*D+d] : V with in-block token index on partitions
        vpk_h = np.ascontiguousarray(
            vctx[:, h].transpose(1, 0, 2).reshape(BS, NBLK * D).astype(bf16)
        )
        qT_h = qg[h].transpose(2, 0, 1).reshape(D, SP).astype(bf16)
        # column order [K(63) | qT | K(0..62)]
        ktq_h = np.ascontiguousarray(
            np.concatenate(
                [ktT_h[:, 63 * BS :], qT_h, ktT_h[:, : 63 * BS]], axis=1
            )
        )
        in_maps.append({"ktT": ktq_h, "vpk": vpk_h, "mask_in": mask})

    trace = bool(int(os.environ.get("BASS_ATTN_TRACE", "0")))
    if trace:
        _install_ntff_hook()
    res = run_bass_kernel_spmd(nc, in_maps, core_ids=list(range(KV)), trace=trace)
    last_exec_time_ns = res.exec_time_ns
    last_profile = res

    out = np.empty((1, S, H * D), dtype=np.float32)
    for h in range(KV):
        oo = res.results[h]["outO"].astype(np.float32)  # [128, O_W]
        oT = oo[:, O_OUT : O_OUT + SP]  # [d, g*128+s], unnormalized
        den = oo[0, O_DEN : O_DEN + SP]  # [g*128+s]
        o = (oT / den[None, :]).reshape(D, G, S).transpose(2, 1, 0)  # [s, g, d]
        out[0, :, h * G * D : (h + 1) * G * D] = o.reshape(S, G * D)
    return out
